# revision 38
# baseline (speedup 1.0000x reference)
import os
from contextlib import ExitStack

import numpy as np
import ml_dtypes

import concourse.bass as bass
import concourse.bacc as bacc
import concourse.mybir as mybir
import concourse.tile as tile
from concourse import bass_utils

F32 = mybir.dt.float32
BF16 = mybir.dt.bfloat16
ALU = mybir.AluOpType
ACTF = mybir.ActivationFunctionType

N_CORES = 8
PPC = 8
NPAIR = PPC // 2
NG = 4
QP = 1
CIN, CMID = 128, 64
S4, S2, POOL = 8, 16, 7
EPS = 1e-5

_POS = np.arange(16, dtype=np.float32) * (np.float32(7.0) / np.float32(15.0))
_LO = np.floor(_POS).astype(np.int32)
_W = (_POS - _LO.astype(np.float32)).astype(np.float32)
for _k in range(1, 15):
    assert _LO[_k] == (_k - 1) // 2, (_k, _LO[_k])
_WODD = _W[1:15:2].copy()
_WDIFF = float(_W[2] - _W[1])
assert np.allclose(_W[2:16:2][:7] - _WODD, _WDIFF, atol=1e-7)

_CPACK_COLS = 7952

_PERM16 = [0] + list(range(1, 15, 2)) + list(range(2, 16, 2)) + [15]
_PERM_X8 = list(range(1, 15, 2)) + [0] + list(range(2, 16, 2)) + [15]


def _upsample_axis(nc, work, tag, src5, dst, dvshape, wov, dt):
    q, o1, seven, o2 = dvshape
    assert seven == 7
    nelem = q * o1 * 7 * o2
    d = work.tile([128, nelem], dt, tag=f"d{tag}", name=f"d{tag}")
    dv = d[:].rearrange("p (q a l b) -> p q a l b", q=q, a=o1, l=7, b=o2)
    nc.vector.tensor_tensor(
        out=dv, in0=src5[:, :, :, 1:8, :], in1=src5[:, :, :, 0:7, :],
        op=ALU.subtract)
    t = work.tile([128, nelem], dt, tag=f"t{tag}", name=f"t{tag}")
    tv = t[:].rearrange("p (q a l b) -> p q a l b", q=q, a=o1, l=7, b=o2)
    nc.vector.tensor_tensor(out=tv, in0=dv, in1=wov, op=ALU.mult)
    ov = dst.rearrange("p (q a k b) -> p q a k b", q=q, a=o1, k=16, b=o2)
    nc.vector.tensor_tensor(
        out=ov[:, :, :, 1:8, :], in0=src5[:, :, :, 0:7, :], in1=tv,
        op=ALU.add)
    nc.vector.tensor_scalar(
        out=d[:], in0=d[:], scalar1=_WDIFF, scalar2=None, op0=ALU.mult)
    nc.vector.tensor_tensor(
        out=ov[:, :, :, 8:15, :], in0=ov[:, :, :, 1:8, :], in1=dv,
        op=ALU.add)
    nc.vector.tensor_copy(ov[:, :, :, 0:16:15, :], src5[:, :, :, 0:8:7, :])
    return ov


def _pool_axis_perm16(nc, work, tag, src5, dst5, shp, dt):
    q, o1, o2 = shp
    m2 = work.tile([128, q * o1 * 8 * o2], dt, tag=f"m2{tag}", name=f"m2{tag}")
    m2v = m2[:].rearrange("p (q a j b) -> p q a j b", q=q, a=o1, j=8, b=o2)
    nc.vector.tensor_tensor(
        out=m2v[:, :, :, 1:7, :], in0=src5[:, :, :, 8:14, :],
        in1=src5[:, :, :, 2:8, :], op=ALU.max)
    nc.vector.tensor_tensor(
        out=m2v[:, :, :, 0:8:7, :], in0=src5[:, :, :, 0:15:14, :],
        in1=src5[:, :, :, 1:16:14, :], op=ALU.max)
    nc.vector.tensor_tensor(
        out=dst5[:, :, :, 0:3, :], in0=m2v[:, :, :, 0:3, :],
        in1=src5[:, :, :, 8:11, :], op=ALU.max)
    nc.vector.tensor_tensor(
        out=dst5[:, :, :, 3:4, :], in0=m2v[:, :, :, 3:4, :],
        in1=m2v[:, :, :, 4:5, :], op=ALU.max)
    nc.vector.tensor_tensor(
        out=dst5[:, :, :, 4:7, :], in0=m2v[:, :, :, 5:8, :],
        in1=src5[:, :, :, 5:8, :], op=ALU.max)


def _pool_axis_permx8(nc, work, tag, src5, dst5, shp, dt):
    q, o1, o2 = shp
    m2 = work.tile([128, q * o1 * 8 * o2], dt, tag=f"m2{tag}", name=f"m2{tag}")
    m2v = m2[:].rearrange("p (q a j b) -> p q a j b", q=q, a=o1, j=8, b=o2)
    nc.vector.tensor_tensor(
        out=m2v[:, :, :, 0:7, :], in0=src5[:, :, :, 7:14, :],
        in1=src5[:, :, :, 0:7, :], op=ALU.max)
    nc.vector.tensor_tensor(
        out=m2v[:, :, :, 7:8, :], in0=src5[:, :, :, 14:15, :],
        in1=src5[:, :, :, 15:16, :], op=ALU.max)
    nc.vector.tensor_tensor(
        out=dst5[:, :, :, 0:3, :], in0=m2v[:, :, :, 0:3, :],
        in1=src5[:, :, :, 8:11, :], op=ALU.max)
    nc.vector.tensor_tensor(
        out=dst5[:, :, :, 3:4, :], in0=m2v[:, :, :, 3:4, :],
        in1=m2v[:, :, :, 4:5, :], op=ALU.max)
    nc.vector.tensor_tensor(
        out=dst5[:, :, :, 4:7, :], in0=m2v[:, :, :, 5:8, :],
        in1=src5[:, :, :, 4:7, :], op=ALU.max)


def _emit_core_program(ctx: ExitStack, tc: "tile.TileContext", aps: dict):
    nc = tc.nc
    fe1, o1p, outd = (aps["fe1"], aps["o1p"], aps["out"])

    consts = ctx.enter_context(tc.tile_pool(name="consts", bufs=1))
    gin = ctx.enter_context(tc.tile_pool(name="gin", bufs=1))
    o1in = ctx.enter_context(tc.tile_pool(name="o1in", bufs=1))
    abufp = ctx.enter_context(tc.tile_pool(name="abufp", bufs=4))
    gbig = ctx.enter_context(tc.tile_pool(name="gbig", bufs=2))
    work = ctx.enter_context(tc.tile_pool(name="work", bufs=1))
    stat = ctx.enter_context(tc.tile_pool(name="stat", bufs=4))
    psy = ctx.enter_context(tc.tile_pool(name="psy", bufs=2, space="PSUM"))

    fbig = gin.tile([128, NPAIR * 1024], BF16, tag="fe1", name="fbig")
    ftiles = [fbig[:, 1024 * p:1024 * (p + 1)] for p in range(NPAIR)]
    nc.sync.dma_start(ftiles[0], fe1[:, 0:1024])
    w1s = consts.tile([128, 256], BF16)
    nc.sync.dma_start(w1s[:], aps["w1t"])
    nc.sync.dma_start(ftiles[1], fe1[:, 1024:2048])
    c3s = consts.tile([128, 512], BF16)
    nc.sync.dma_start(c3s[:], aps["c3"])
    wx8s = consts.tile([128, 8], BF16)
    nc.sync.dma_start(wx8s[:], aps["wx8"])
    affs = consts.tile([128, 8], F32)
    nc.sync.dma_start(affs[:], aps["aff"])
    nc.sync.dma_start(fbig[:, 2048:4096], fe1[:, 2048:4096])
    wyos = consts.tile([128, 112], BF16)
    nc.sync.dma_start(wyos[:], aps["wyo"])
    wzo = consts.tile([128, 1792], BF16)
    nc.sync.dma_start(wzo[:], aps["wzo"])
    wb1s = consts.tile([128, 128], BF16)
    nc.sync.dma_start(wb1s[:], aps["wb1t"])
    wb2s = consts.tile([128, 128], BF16)
    nc.sync.dma_start(wb2s[:], aps["wb2t"])
    o1big = o1in.tile([128, 4 * 4096], BF16, tag="o1", name="o1big")
    nc.sync.dma_start(o1big[:], o1p)
    o1tiles = [o1big[:, 4096 * p:4096 * (p + 1)] for p in range(NPAIR)]
    g2c, be2c, epsc = (affs[:, i:i + 1] for i in (2, 3, 4))

    dummy = stat.tile([128, 1], F32, tag="dummy", name="dummy")
    nc.scalar.activation(dummy[:], affs[:, 4:5], ACTF.Sqrt)
    nc.scalar.activation(dummy[:], affs[:, 4:5], ACTF.Identity)

    psAs, nm1s, abufs = [None] * NPAIR, [None] * NPAIR, [None] * NPAIR
    uzs = [None] * NG
    inv1s, ss1s = [None] * NPAIR, [None] * NPAIR
    wb1qs = [None] * NPAIR
    syqs, ss2s = [None] * NPAIR, [None] * NPAIR
    sc2s, bi2s = [None] * NPAIR, [None] * NPAIR
    std2s, m2ss = [None] * NPAIR, [None] * NPAIR
    pzms = [None] * NPAIR

    def emit_mm1(p):
        psAt = psy.tile([128, 2048], F32, tag="psY", name="psY")
        psA = psAt[:, 0:512]
        nc.tensor.matmul(psA, w1s[:, 0:128], ftiles[p][:, 0:512],
                         start=True, stop=False)
        nc.tensor.matmul(psA, w1s[:, 128:256], ftiles[p][:, 512:1024],
                         start=False, stop=True)
        psAs[p] = psA

    def emit_c3(p):
        nm1 = stat.tile([128, 1], F32, tag="nm1", name="nm1")
        scr512 = work.tile([128, 512], F32, tag="scr512", name="scr512")
        nc.vector.scalar_tensor_tensor(
            out=scr512[:], in0=psAs[p], scalar=-1.0, in1=c3s[:],
            op0=ALU.mult, op1=ALU.mult, accum_out=nm1[:])
        nm1s[p] = nm1

    def emit_acopy(p):
        abuf = abufp.tile([128, 512], BF16, tag="abuf", name="abuf")
        nc.scalar.activation(abuf[:], psAs[p], ACTF.Identity,
                             bias=nm1s[p][:], scale=1.0)
        abufs[p] = abuf

    upstate = {}

    def emit_up_x(g):
        q = QP
        d8 = work.tile([128, q * 64 * 8], BF16, tag="d8", name="d8")
        d8v = d8[:].rearrange("p (q a l) -> p q a l", q=q, a=64, l=8)
        nc.vector.memset(d8v[:, :, :, 7:8], 0.0)
        for i in range(q):
            av = abufs[QP * g + i][:].rearrange(
                "p (a l) -> p a l", a=64, l=8)
            nc.vector.tensor_tensor(
                out=d8v[:, i, :, 0:7], in0=av[:, :, 1:8], in1=av[:, :, 0:7],
                op=ALU.subtract)
        tw8 = work.tile([128, q * 64 * 8], BF16, tag="tw8", name="tw8")
        wx8b = wx8s[:].unsqueeze(1).broadcast_to([128, q * 64, 8])
        nc.vector.tensor_tensor(
            out=tw8[:].rearrange("p (a l) -> p a l", a=q * 64, l=8),
            in0=d8[:].rearrange("p (a l) -> p a l", a=q * 64, l=8),
            in1=wx8b, op=ALU.mult)
        ux = work.tile([128, q * 1024], BF16, tag="ux", name="ux")
        uxv = ux[:].rearrange("p (q a k) -> p q a k", q=q, a=64, k=16)
        tw8v = tw8[:].rearrange("p (q a l) -> p q a l", q=q, a=64, l=8)
        for i in range(q):
            av = abufs[QP * g + i][:].rearrange(
                "p (a l) -> p a l", a=64, l=8)
            nc.vector.tensor_tensor(
                out=uxv[:, i, :, 0:8], in0=av, in1=tw8v[:, i], op=ALU.add)
        nc.vector.tensor_scalar(
            out=d8[:], in0=d8[:], scalar1=_WDIFF, scalar2=None, op0=ALU.mult)
        nc.vector.tensor_tensor(
            out=uxv[:, :, :, 8:16], in0=uxv[:, :, :, 0:8], in1=d8v,
            op=ALU.add)
        for i in range(q):
            av = abufs[QP * g + i][:].rearrange(
                "p (a l) -> p a l", a=64, l=8)
            nc.vector.tensor_copy(uxv[:, i, :, 7:8], av[:, :, 0:1])
        upstate[("ux", g)] = ux

    def emit_up_y(g):
        q = QP
        ux = upstate[("ux", g)]
        ux5 = ux[:].rearrange("p (q z y x) -> p q z y x", q=q, z=8, y=8, x=16)
        uy = work.tile([128, q * 2048], BF16, tag="uy", name="uy")
        wyob = (wyos[:].rearrange("p (l b) -> p l b", l=7, b=16)
                .unsqueeze(1).unsqueeze(1).broadcast_to([128, q, 8, 7, 16]))
        _upsample_axis(nc, work, "y", ux5, uy[:], (q, 8, 7, 16), wyob, BF16)
        upstate[("uy", g)] = uy

    def emit_up_z(g):
        q = QP
        uy = upstate[("uy", g)]
        uy5 = uy[:].rearrange("p (q a z s) -> p q a z s", q=q, a=1, z=8, s=256)
        uz = gbig.tile([128, q * 4096], BF16, tag="uz", name="uz", bufs=3)
        wzob = wzo[:].rearrange("p (q a l b) -> p q a l b", q=1, a=1, l=7, b=256)
        _upsample_axis(nc, work, "z", uy5, uz[:], (q, 1, 7, 256), wzob, BF16)
        uzs[g] = uz

    def emit_sq1(p):
        g, i = divmod(p, QP)
        ss1 = stat.tile([128, 1], F32, tag="ss1", name="ss1")
        scr = work.tile([128, 4096], BF16, tag="scr", name="scr")
        nc.scalar.activation(
            scr[:], uzs[g][:, 4096 * i:4096 * (i + 1)], ACTF.Square,
            accum_out=ss1[:])
        ss1s[p] = ss1

    rbufs = [None] * NPAIR

    def emit_relu(p):
        rb = work.tile([128, 4096], BF16, tag="rbuf", name="rbuf", bufs=2)
        nc.vector.tensor_scalar(
            out=rb[:], in0=uzs[p][:], scalar1=0.0, scalar2=None, op0=ALU.max)
        rbufs[p] = rb

    std1s = [None] * NPAIR

    def emit_stats1a(p):
        var1 = stat.tile([128, 1], F32, tag="var1", name="var1")
        nc.vector.tensor_scalar(
            out=var1[:], in0=ss1s[p][:], scalar1=1.0 / 4096.0, scalar2=None,
            op0=ALU.mult)
        std1 = stat.tile([128, 1], F32, tag="std1", name="std1")
        nc.scalar.activation(std1[:], var1[:], ACTF.Sqrt, bias=epsc)
        std1s[p] = std1

    def emit_stats1b(p):
        inv1 = stat.tile([128, 1], F32, tag="inv1", name="inv1")
        nc.vector.reciprocal(inv1[:], std1s[p][:])
        inv1s[p] = inv1
        wb1q = stat.tile([128, 128], BF16, tag="wb1q", name="wb1q")
        nc.vector.tensor_scalar(
            out=wb1q[:], in0=wb1s[:], scalar1=inv1[:], scalar2=None,
            op0=ALU.mult)
        wb1qs[p] = wb1q

    _CHUNKS = [(0, 2048), (2048, 2048)]

    def emit_mm2(p):
        g, i = divmod(p, 2)
        r = rbufs[p][:]
        if i == 0:
            upstate[("ybfg", g)] = gbig.tile(
                [128, 2 * 4096], BF16, tag="ybfg", name="ybfg")
        ybf = upstate[("ybfg", g)][:, 4096 * i:4096 * (i + 1)]
        syq = stat.tile([128, 2], F32, tag="syq", name="syq")
        ss2 = stat.tile([128, 2], F32, tag="ss2", name="ss2")
        for (c0, clen) in _CHUNKS:
            psY = psy.tile([128, 2048], F32, tag="psY", name="psY")
            for jj in range(0, clen, 512):
                nc.tensor.matmul(psY[:, jj:jj + 512],
                                 wb1qs[p][:], r[:, c0 + jj:c0 + jj + 512],
                                 start=True, stop=False)
            for jj in range(0, clen, 512):
                nc.tensor.matmul(psY[:, jj:jj + 512],
                                 wb2s[:], o1tiles[p][:, c0 + jj:c0 + jj + 512],
                                 start=False, stop=True)
            ci = c0 // 2048
            nc.scalar.activation(
                ybf[:, c0:c0 + clen], psY[:, 0:clen], ACTF.Identity,
                accum_out=syq[:, ci:ci + 1])
        syqs[p] = syq
        ss2s[p] = ss2

    def emit_sqy3(p):
        g, i = divmod(p, 2)
        ybf = upstate[("ybfg", g)][:, 4096 * i:4096 * (i + 1)]
        for (c0, clen) in _CHUNKS:
            ci = c0 // 2048
            scr = work.tile([128, 4096], BF16, tag="scr", name="scr")
            nc.scalar.activation(
                scr[:, 0:clen], ybf[:, c0:c0 + clen], ACTF.Square,
                accum_out=ss2s[p][:, ci:ci + 1])

    def emit_stats2a(p):
        sy = stat.tile([128, 1], F32, tag="sy", name="sy")
        nc.vector.tensor_tensor(
            out=sy[:], in0=syqs[p][:, 0:1], in1=syqs[p][:, 1:2], op=ALU.add)
        mean2 = stat.tile([128, 1], F32, tag="mean2", name="mean2")
        nc.vector.tensor_scalar(
            out=mean2[:], in0=sy[:], scalar1=1.0 / 4096.0, scalar2=None,
            op0=ALU.mult)
        ssa = stat.tile([128, 1], F32, tag="ssa", name="ssa")
        nc.vector.tensor_tensor(
            out=ssa[:], in0=ss2s[p][:, 0:1], in1=ss2s[p][:, 1:2], op=ALU.add)
        ey2 = stat.tile([128, 1], F32, tag="ey2", name="ey2")
        nc.vector.tensor_scalar(
            out=ey2[:], in0=ssa[:], scalar1=1.0 / 4096.0, scalar2=None,
            op0=ALU.mult)
        m2s = stat.tile([128, 1], F32, tag="m2s", name="m2s")
        nc.vector.tensor_tensor(out=m2s[:], in0=mean2[:], in1=mean2[:],
                                op=ALU.mult)
        var2 = stat.tile([128, 1], F32, tag="var2", name="var2")
        nc.vector.tensor_tensor(out=var2[:], in0=ey2[:], in1=m2s[:],
                                op=ALU.subtract)
        std2 = stat.tile([128, 1], F32, tag="std2", name="std2")
        nc.scalar.activation(std2[:], var2[:], ACTF.Sqrt, bias=epsc)
        std2s[p] = std2
        m2ss[p] = mean2

    def emit_stats2b(p):
        mean2 = m2ss[p]
        inv2 = stat.tile([128, 1], F32, tag="inv2", name="inv2")
        nc.vector.reciprocal(inv2[:], std2s[p][:])
        scale2 = stat.tile([128, 1], F32, tag="scale2", name="scale2")
        nc.vector.tensor_scalar(
            out=scale2[:], in0=inv2[:], scalar1=g2c, scalar2=None,
            op0=ALU.mult)
        nm2 = stat.tile([128, 1], F32, tag="nm2", name="nm2")
        nc.vector.tensor_scalar(
            out=nm2[:], in0=mean2[:], scalar1=-1.0, scalar2=None, op0=ALU.mult)
        tb2 = stat.tile([128, 1], F32, tag="tb2", name="tb2")
        nc.vector.tensor_tensor(out=tb2[:], in0=nm2[:], in1=scale2[:],
                                op=ALU.mult)
        bias2 = stat.tile([128, 1], F32, tag="bias2", name="bias2")
        nc.vector.tensor_scalar(
            out=bias2[:], in0=tb2[:], scalar1=be2c, scalar2=None, op0=ALU.add)
        sc2s[p] = scale2
        bi2s[p] = bias2

    def emit_pool_final(g):
        q = 2
        ybfg = upstate[("ybfg", g)]
        yv = ybfg[:].rearrange("p (q a z s) -> p q a z s",
                               q=q, a=1, z=16, s=256)
        pz1 = work.tile([128, q * 7 * 256], BF16, tag="pz1", name="pz1")
        pzv1 = pz1[:].rearrange("p (q a z s) -> p q a z s",
                                q=q, a=1, z=7, s=256)
        _pool_axis_perm16(nc, work, "z", yv, pzv1, (q, 1, 256), BF16)
        pzy = pz1[:].rearrange("p (q z y x) -> p q z y x",
                               q=q, z=7, y=16, x=16)
        py = work.tile([128, q * 7 * 7 * 16], BF16, tag="py", name="py")
        pyv = py[:].rearrange("p (q z y x) -> p q z y x",
                              q=q, z=7, y=7, x=16)
        _pool_axis_perm16(nc, work, "y", pzy, pyv, (q, 7, 16), BF16)
        pyx = py[:].rearrange("p (q a x o) -> p q a x o",
                              q=q, a=49, x=16, o=1)
        pzm = work.tile([128, q * 343], BF16, tag="pzm", name="pzm")
        pzv = pzm[:].rearrange("p (q a x o) -> p q a x o",
                               q=q, a=49, x=7, o=1)
        _pool_axis_permx8(nc, work, "x", pyx, pzv, (q, 49, 1), BF16)
        for i in range(q):
            pzms[2 * g + i] = pzm[:, 343 * i:343 * (i + 1)]

    def emit_pool1(p):
        g, i = divmod(p, 2)
        ybf = upstate[("ybfg", g)][:, 4096 * i:4096 * (i + 1)]
        yv = ybf.rearrange("p (q a z s) -> p q a z s", q=1, a=1, z=16, s=256)
        pz1 = work.tile([128, 7 * 256], BF16, tag="pz1b", name="pz1b")
        pzv1 = pz1[:].rearrange("p (q a z s) -> p q a z s",
                                q=1, a=1, z=7, s=256)
        _pool_axis_perm16(nc, work, "zb", yv, pzv1, (1, 1, 256), BF16)
        pzy = pz1[:].rearrange("p (q z y x) -> p q z y x",
                               q=1, z=7, y=16, x=16)
        py = work.tile([128, 7 * 7 * 16], BF16, tag="pyb", name="pyb")
        pyv = py[:].rearrange("p (q z y x) -> p q z y x",
                              q=1, z=7, y=7, x=16)
        _pool_axis_perm16(nc, work, "yb", pzy, pyv, (1, 7, 16), BF16)
        pyx = py[:].rearrange("p (q a x o) -> p q a x o",
                              q=1, a=49, x=16, o=1)
        pzm = work.tile([128, 343], BF16, tag="pzmb", name="pzmb")
        pzv = pzm[:].rearrange("p (q a x o) -> p q a x o",
                               q=1, a=49, x=7, o=1)
        _pool_axis_permx8(nc, work, "xb", pyx, pzv, (1, 49, 1), BF16)
        pzms[p] = pzm[:]

    def emit_final1(p):
        outt = work.tile([128, 343], F32, tag="outt", name="outt", bufs=2)
        nc.scalar.activation(
            outt[:], pzms[p], ACTF.Relu, bias=bi2s[p][:], scale=sc2s[p][:])
        nc.sync.dma_start(outd[128 * p:128 * (p + 1), :], outt[:])

    emit_mm1(0)
    emit_c3(0)
    emit_acopy(0)
    emit_mm1(1)
    emit_c3(1)
    emit_acopy(1)
    emit_up_x(0)
    emit_up_y(0)
    emit_mm1(2)
    emit_c3(2)
    emit_acopy(2)
    emit_up_z(0)
    emit_sq1(0)
    emit_relu(0)
    emit_up_x(1)
    emit_stats1a(0)
    emit_up_y(1)
    emit_mm1(3)
    emit_c3(3)
    emit_acopy(3)
    emit_stats1b(0)
    emit_mm2(0)
    emit_up_z(1)
    emit_sq1(1)
    emit_relu(1)
    emit_stats1a(1)
    emit_up_x(2)
    emit_stats1b(1)
    emit_mm2(1)
    emit_up_y(2)
    emit_sqy3(0)
    emit_stats2a(0)
    emit_up_z(2)
    emit_sq1(2)
    emit_relu(2)
    emit_stats1a(2)
    emit_up_x(3)
    emit_stats1b(2)
    emit_mm2(2)
    emit_sqy3(1)
    emit_stats2a(1)
    emit_up_y(3)
    emit_up_z(3)
    emit_sq1(3)
    emit_relu(3)
    emit_stats1a(3)
    emit_stats1b(3)
    emit_mm2(3)
    emit_pool_final(0)
    emit_stats2b(0)
    emit_stats2b(1)
    emit_final1(0)
    emit_final1(1)
    emit_sqy3(2)
    emit_stats2a(2)
    emit_pool1(2)
    emit_sqy3(3)
    emit_pool1(3)
    emit_stats2a(3)
    emit_stats2b(2)
    emit_final1(2)
    emit_stats2b(3)
    emit_final1(3)


def build_program():
    nc = bacc.Bacc("TRN2", target_bir_lowering=False, debug=False)
    aps = {
        "fe1": nc.dram_tensor("fe1", [128, NPAIR * 1024], BF16, kind="ExternalInput").ap(),
        "o1p": nc.dram_tensor("o1p", [128, NPAIR * 4096], BF16, kind="ExternalInput").ap(),
        "w1t": nc.dram_tensor("w1t", [128, 256], BF16, kind="ExternalInput").ap(),
        "wb1t": nc.dram_tensor("wb1t", [128, 128], BF16, kind="ExternalInput").ap(),
        "wb2t": nc.dram_tensor("wb2t", [128, 128], BF16, kind="ExternalInput").ap(),
        "c3": nc.dram_tensor("c3", [128, 512], BF16, kind="ExternalInput").ap(),
        "wx8": nc.dram_tensor("wx8", [128, 8], BF16, kind="ExternalInput").ap(),
        "wyo": nc.dram_tensor("wyo", [128, 112], BF16, kind="ExternalInput").ap(),
        "wzo": nc.dram_tensor("wzo", [128, 1792], BF16, kind="ExternalInput").ap(),
        "aff": nc.dram_tensor("aff", [128, 8], F32, kind="ExternalInput").ap(),
        "out": nc.dram_tensor("out", [PPC * 64, 343], F32, kind="ExternalOutput").ap(),
    }
    with tile.TileContext(nc) as tc, ExitStack() as ctx:
        _emit_core_program(ctx, tc, aps)
    nc.compile()
    return nc


def _resize_colsums():
    u = np.zeros((16, 8), np.float64)
    for k in range(16):
        lo = int(_LO[k])
        hi = min(lo + 1, 7)
        w = float(_W[k])
        u[k, lo] += 1.0 - w
        u[k, hi] += w
    return u.sum(axis=0)


def shard_inputs(inputs: dict) -> list[dict]:
    comb2 = np.ascontiguousarray(np.asarray(inputs["comb2"], dtype=np.float32))
    out1 = np.ascontiguousarray(np.asarray(inputs["out1"], dtype=np.float32))
    props = np.asarray(inputs["proposals"]).astype(np.int64)
    w_up2 = np.asarray(inputs["w_up2"], dtype=np.float32)
    w_back2 = np.asarray(inputs["w_back2"], dtype=np.float32)
    g1 = np.asarray(inputs["g_up2"], dtype=np.float32)
    be1 = np.asarray(inputs["be_up2"], dtype=np.float32)
    g2 = np.asarray(inputs["g_back2"], dtype=np.float32)
    be2 = np.asarray(inputs["be_back2"], dtype=np.float32)
    bf = ml_dtypes.bfloat16
    assert np.all(np.asarray(be1) == 0.0) and np.all(np.asarray(g1) > 0.0), (
        "kernel relies on be_up2 == 0 and g_up2 > 0")

    B = comb2.shape[0]
    d4 = comb2.shape[2]
    d2 = out1.shape[2]

    w1t = np.zeros((128, 256), np.float32)
    w1t[:, 0:64] = w_up2.T
    w1t[:, 128 + 64:256] = w_up2.T
    wb1g = w_back2[:, 0:64] * g1[None, :]
    wb1tt = np.zeros((128, 128), np.float32)
    wb1tt[0:64, 0:64] = wb1g.T
    wb1tt[64:128, 64:128] = wb1g.T
    wb2t = np.zeros((128, 128), np.float32)
    wb2t[0:64, 0:64] = w_back2[:, 64:128].T
    wb2t[64:128, 64:128] = w_back2[:, 64:128].T

    c = _resize_colsums()
    c3row = (np.einsum("i,j,k->ijk", c, c, c).reshape(512) / 4096.0)
    w8 = np.concatenate([_WODD, [np.float32(0.0)]])
    aff = np.zeros((128, 8), np.float32)
    aff[:, 2] = np.concatenate([g2, g2])
    aff[:, 3] = np.concatenate([be2, be2])
    aff[:, 4] = EPS

    common = dict(
        w1t=w1t.astype(bf), wb1t=wb1tt.astype(bf), wb2t=wb2t.astype(bf),
        c3=np.tile(c3row.astype(bf), (128, 1)),
        wx8=np.tile(w8.astype(bf), (128, 1)),
        wyo=np.tile(np.repeat(_WODD, 16).astype(bf), (128, 1)),
        wzo=np.tile(np.tile(np.repeat(_WODD, 256), 1).astype(bf), (128, 1)),
        aff=aff)

    in_maps = []
    for cidx in range(N_CORES):
        fe1 = np.empty((128, NPAIR * 1024), bf)
        o1p = np.empty((128, NPAIR * 4096), bf)
        for pr in range(NPAIR):
            for s in range(2):
                i = PPC * cidx + 2 * pr + s
                b = int(props[i, 0]) % B
                z4 = min(max(int(props[i, 1]) // 4, 0), d4 - S4)
                y4 = min(max(int(props[i, 2]) // 4, 0), d4 - S4)
                x4 = min(max(int(props[i, 3]) // 4, 0), d4 - S4)
                fe1[:, 1024 * pr + 512 * s:1024 * pr + 512 * (s + 1)] = (
                    comb2[b, :, z4:z4 + S4, y4:y4 + S4, x4:x4 + S4]
                    .reshape(CIN, 512).astype(bf))
                z2 = min(max(int(props[i, 1]) // 2, 0), d2 - S2)
                y2 = min(max(int(props[i, 2]) // 2, 0), d2 - S2)
                x2 = min(max(int(props[i, 3]) // 2, 0), d2 - S2)
                crop = out1[b, :, z2:z2 + S2, y2:y2 + S2, x2:x2 + S2]
                crop = crop[:, _PERM16][:, :, _PERM16][:, :, :, _PERM_X8]
                o1p[64 * s:64 * (s + 1), 4096 * pr:4096 * (pr + 1)] = (
                    crop.reshape(CMID, 4096).astype(bf))
        in_maps.append(dict(fe1=fe1, o1p=o1p, **common))
    return in_maps


_CACHE = {}


def _get_program():
    if "nc" not in _CACHE:
        _CACHE["nc"] = build_program()
    return _CACHE["nc"]


def kernel(**inputs) -> np.ndarray:
    nc = _get_program()
    in_maps = shard_inputs(inputs)
    res = bass_utils.run_bass_kernel_spmd(
        nc, in_maps, core_ids=list(range(N_CORES)),
    )
    if res.exec_time_ns is not None:
        print(f"HW exec time: {res.exec_time_ns} ns")
    outs = [r["out"].reshape(PPC, 64, 7, 7, 7) for r in res.results]
    return np.concatenate(outs, axis=0)
